# revision 8
# baseline (speedup 1.0000x reference)
"""ForwardWarpDWeight (bilinear splat forward warp) on 8 trn2 NeuronCores.

Pure data parallel per the sharding hint: batch element b runs on core b;
each splat is independent per batch element so there is no cross-device
scatter traffic. The warp (clip, depth-weight exp, bilinear corner
weights + validity, fused 5-channel scatter-add splat, normalize) runs
on-device, sharded over the 8 cores with shard_map.

Note: a hand-written Bass pipeline (DVE elementwise + CCE-add indirect-DMA
scatter into engine-private HBM accumulators) was prototyped first, but this
container's Q7 vector-indirect descriptor generator mispairs idx<->payload
for multi-index-per-partition scatters (payloads stream from the first
index instead of honoring per-descriptor addresses), which corrupts any
per-pixel scatter. The splat is therefore issued through the device
compiler's scatter-add path, which is correct on this hardware.
"""
import os
import sys

import numpy as np

B, C, H, W = 8, 3, 384, 1280
REF_SCALE = 5.0

_JITTED = None


def _build():
    global _JITTED
    if _JITTED is not None:
        return _JITTED
    import jax
    import jax.numpy as jnp
    from jax.sharding import Mesh, PartitionSpec
    from jax.experimental.shard_map import shard_map

    def _forward_warp(im, flow):
        # im: [b,Ch,H,W], flow: [b,H,W,2]
        b, Ch, Hh, Ww = im.shape
        xs = flow[..., 0] + jnp.arange(Ww, dtype=flow.dtype)
        ys = flow[..., 1] + jnp.arange(Hh, dtype=flow.dtype)[:, None]
        x0 = jnp.floor(xs); y0 = jnp.floor(ys)
        x1 = x0 + 1.0;      y1 = y0 + 1.0
        valid = (x0 >= 0) & (x1 <= Ww - 1) & (y0 >= 0) & (y1 <= Hh - 1)
        x0i = x0.astype(jnp.int32); y0i = y0.astype(jnp.int32)
        x1i = x1.astype(jnp.int32); y1i = y1.astype(jnp.int32)
        w_nw = (x1 - xs) * (y1 - ys)
        w_ne = (xs - x0) * (y1 - ys)
        w_sw = (x1 - xs) * (ys - y0)
        w_se = (xs - x0) * (ys - y0)

        def splat_one(im_b, idx_b, w_b):
            contrib = (im_b * w_b).reshape(Ch, -1).T
            return jax.ops.segment_sum(contrib, idx_b.reshape(-1),
                                       num_segments=Hh * Ww)

        out = jnp.zeros((b, Hh * Ww, Ch), dtype=im.dtype)
        for w_c, yi, xi in ((w_nw, y0i, x0i), (w_ne, y0i, x1i),
                            (w_sw, y1i, x0i), (w_se, y1i, x1i)):
            idx = jnp.where(valid, yi * Ww + xi, 0)
            wv = jnp.where(valid, w_c, jnp.zeros_like(w_c))
            out = out + jax.vmap(splat_one)(im, idx, wv)
        return out.transpose(0, 2, 1).reshape(b, Ch, Hh, Ww)

    def _per_shard(x, flow, depth):
        # local shapes: x [1,3,H,W], flow [1,2,H,W], depth [1,1,H,W]
        flow = jnp.clip(flow, -2.0 * W, 2.0 * W)
        flow = jnp.transpose(flow, (0, 2, 3, 1))
        depth = jnp.clip(depth, 0.001, 80.0)
        depth_weight = jnp.exp(-(depth - 40.0) / REF_SCALE)
        mask = jnp.ones_like(depth)
        stacked = jnp.concatenate([depth_weight, x * depth_weight, mask], axis=1)
        warped = _forward_warp(stacked, flow)
        dw_flowed = warped[:, 0:1]
        xw_flowed = warped[:, 1:1 + C]
        mask_flowed = warped[:, 1 + C:2 + C]
        invalid = mask_flowed < 0.5
        xw_flowed = jnp.where(invalid, jnp.zeros_like(xw_flowed), xw_flowed)
        return xw_flowed / jnp.maximum(dw_flowed, 1e-7)

    devices = jax.devices()[:B]
    mesh = Mesh(np.asarray(devices), ("b",))
    fn = jax.jit(
        shard_map(
            _per_shard,
            mesh=mesh,
            in_specs=(PartitionSpec("b"), PartitionSpec("b"), PartitionSpec("b")),
            out_specs=PartitionSpec("b"),
            check_rep=False,
        )
    )
    _JITTED = fn
    return fn


def kernel(x, flow, depth):
    import jax

    fn = _build()
    x = np.ascontiguousarray(x, dtype=np.float32)
    flow = np.ascontiguousarray(flow, dtype=np.float32)
    depth = np.ascontiguousarray(depth, dtype=np.float32)
    out = fn(x, flow, depth)
    out = np.asarray(jax.block_until_ready(out))
    return out.astype(np.float32)
